# revision 29
# baseline (speedup 1.0000x reference)
"""Trainium2 Bass kernel for nn_Asterisk: 4-branch directional dilated conv.

Math: every branch is a sum of 5 shifted 1x1 convolutions over C=256:
  y1[i,j] = sum_m Wh[m]  . x[i,      j+2m]   (horizontal)
  y2[i,j] = sum_m Wv[m]  . x[i+2m,   j]      (vertical)
  y3[i,j] = sum_m Wd1[m] . x[i+2m, j-2m]     (shear-diag)
  y4[i,j] = sum_m Wd2[2-m]. x[i+2m, j-2m]    (shear-antidiag; same taps as y3)
All shifts are zero-padded at image boundaries.

Strategy: data-parallel over batch (B=8 -> 8 NeuronCores, zero collectives).
Per core: x[b] lives in SBUF as two 128-channel chunks, rows zero-padded by
4 top/bottom (memset) and columns zero-padded by 4 (host-side, so every DMA
stays contiguous) -- shifted taps then need no boundary handling at all.
Each output tile = 4 rows x 128 cols (N=512, one PSUM bank),
accumulated via matmuls over (tap, chunk). y3/y4 share rhs tiles so their
weights concat to full M=128; y1/y2 share only the center tap.
Compute dtype bf16 (host-cast), accumulation fp32.
"""
import os
import sys

for _p in ("/opt/trn_rl_repo",):
    if _p not in sys.path:
        sys.path.insert(0, _p)

import numpy as np
import ml_dtypes

import concourse.bass as bass
import concourse.mybir as mybir
from concourse.bass_utils import run_bass_kernel_spmd

B, C, H, W = 8, 256, 128, 128
OC = C // 4          # 64 output channels per branch
NCORES = 8
RP = 4               # row padding (max |shift|)
CP = 4               # column padding (host-side, in DRAM)
HP = H + 2 * RP      # padded rows in SBUF
WP = W + 2 * CP      # padded cols (host-padded)
TR = 4               # output rows per tile
NT = H // TR         # 32 row-tiles
NPSUM = 8            # psum banks in flight
NSTAGE = 4           # sbuf staging buffers for output

DT = mybir.dt.bfloat16
NPDT = ml_dtypes.bfloat16

LAST_EXEC_TIME_NS = None
LAST_RESULTS = None

_cached_nc = None


def _build_weights(w_h, b_h, w_v, b_v, w_d1, b_d1, w_d2, b_d2):
    """Build the stationary matmul operands.

    Returns:
      w128: [128, 12, 128]  (c-within-chunk, tile, M) tiles with M=128:
            t=0,1: center tap [y1|y2] for chunk k=t
            t=2+2*(m+2)+k: diag tap m for chunk k, [y3|y4]
      w64:  [128, 16, 64] tiles with M=64:
            t=2*mi+k: y1 edge tap (m in -2,-1,1,2), chunk k
            t=8+2*mi+k: y2 edge tap, chunk k
      bias: [128, 2] f32, col0=[b_h;b_v], col1=[b_d1;b_d2]
    """
    ms_edge = (-2, -1, 1, 2)
    w128 = np.zeros((128, 12, 128), np.float32)
    w64 = np.zeros((128, 16, 64), np.float32)
    for k in range(2):
        cs = slice(128 * k, 128 * (k + 1))
        # center: y1 tap m=0 is w_h[..., 2]; y2 tap m=0 is w_v[..., 2, 0]
        w128[:, k, 0:64] = w_h[:, cs, 0, 2].T
        w128[:, k, 64:128] = w_v[:, cs, 2, 0].T
        for mi, m in enumerate((-2, -1, 0, 1, 2)):
            t = 2 + 2 * (m + 2) + k
            w128[:, t, 0:64] = w_d1[:, cs, m + 2, 0].T
            w128[:, t, 64:128] = w_d2[:, cs, 0, 2 - m].T
        for mi, m in enumerate(ms_edge):
            w64[:, 2 * mi + k, :] = w_h[:, cs, 0, m + 2].T
            w64[:, 8 + 2 * mi + k, :] = w_v[:, cs, m + 2, 0].T
    bias = np.stack(
        [np.concatenate([b_h, b_v]), np.concatenate([b_d1, b_d2])], axis=1
    ).astype(np.float32)
    return w128, w64, bias


def _build_nc():
    nc = bass.Bass()
    x_ext = nc.declare_dram_parameter("x", [C, H, WP], DT, isOutput=False)
    w128_ext = nc.declare_dram_parameter("w128", [128, 12, 128], DT, isOutput=False)
    w64_ext = nc.declare_dram_parameter("w64", [128, 16, 64], DT, isOutput=False)
    bias_ext = nc.declare_dram_parameter("bias", [128, 2], mybir.dt.float32, isOutput=False)
    out_ext = nc.declare_dram_parameter("out", [C, H, W], mybir.dt.float32, isOutput=True)

    xsb = [nc.alloc_sbuf_tensor(f"x{k}", [128, HP, WP], DT).ap() for k in range(2)]
    w128_sb = nc.alloc_sbuf_tensor("w128s", [128, 12, 128], DT).ap()
    w64_sb = nc.alloc_sbuf_tensor("w64s", [128, 16, 64], DT).ap()
    bias_sb = nc.alloc_sbuf_tensor("biass", [128, 2], mybir.dt.float32).ap()
    stage = [
        nc.alloc_sbuf_tensor(f"st{i}", [128, TR * W], mybir.dt.float32).ap()
        for i in range(NSTAGE)
    ]
    psum = [
        nc.alloc_psum_tensor(f"p{i}", [128, TR * W], mybir.dt.float32).ap()
        for i in range(NPSUM)
    ]
    warm = nc.alloc_sbuf_tensor("warm", [128, 512], DT).ap()

    # input stripes (row0, nrows): fine-grained early so compute starts ASAP
    STRIPES = [(0, 4), (4, 4), (8, 8), (16, 16), (32, 16), (48, 16), (64, 32), (96, 32)]
    NSTRIPE = len(STRIPES)
    row2stripe = {}
    for si, (sr0, sn) in enumerate(STRIPES):
        for r in range(sr0, sr0 + sn):
            row2stripe[r] = si
    NWARM = 16  # PE warm-up matmuls (HAM ramp + queue priming) during DMA wait

    # group list: g = 2*t + br, br 0 => [y1|y2] channels 0:128, 1 => [y3|y4]
    groups = [(t, br) for t in range(NT) for br in (0, 1)]

    import contextlib
    with contextlib.ExitStack() as _stack:
        block = _stack.enter_context(nc.Block())
        w_sem = _stack.enter_context(nc.semaphore("w_sem"))
        ms_sem = _stack.enter_context(nc.semaphore("ms_sem"))
        mm_sem = _stack.enter_context(nc.semaphore("mm_sem"))
        act_sem = _stack.enter_context(nc.semaphore("act_sem"))
        x_sems = [_stack.enter_context(nc.semaphore(f"x_sem{s}")) for s in range(NSTRIPE)]
        o_sems = [_stack.enter_context(nc.semaphore(f"o_sem{i}")) for i in range(NSTAGE)]

        @block.sync
        def _(sync):
            for s, (r0, nr) in enumerate(STRIPES):
                for k in range(2):
                    sync.dma_start(
                        xsb[k][:, RP + r0 : RP + r0 + nr, :],
                        x_ext[128 * k : 128 * (k + 1), r0 : r0 + nr, :],
                    ).then_inc(x_sems[s], 16)

        @block.vector
        def _(vector):
            vector.memset(warm[:, :], 0.0).then_inc(ms_sem, 1)
            for k in range(2):
                vector.memset(xsb[k][:, 0:RP, :], 0.0).then_inc(ms_sem, 1)
                vector.memset(xsb[k][:, RP + H : HP, :], 0.0).then_inc(ms_sem, 1)

        @block.tensor
        def _(tensor):
            tensor.wait_ge(ms_sem, 1)
            # Warm-up: keeps the PE HAM busy during the input-DMA wait so the
            # real matmuls all run at 2.4 GHz; results are never read.
            for wi in range(NWARM):
                tensor.matmul(psum[wi % NPSUM][:, :], lhsT=warm[:, 0:128], rhs=warm[:, :],
                              start=True, stop=True)
            tensor.wait_ge(ms_sem, 5)
            tensor.wait_ge(w_sem, 48)

            def win_f(r0):
                def win(k, di, dj):
                    return xsb[k][:, r0 + di : r0 + di + TR, CP + dj : CP + dj + W]
                return win

            def group_mms(t, br):
                # (out_ap, lhsT_ap, rhs_ap, start, stop). First and last
                # matmul of each group are full-coverage (PSUM accumulation
                # group start/stop applies to whole byte ranges): center-tap
                # chunk 0 opens with start=True, chunk 1 closes with stop=True.
                win = win_f(RP + 4 * t)
                mms = []
                if br == 0:
                    mms.append((P_cur[:, :], w128_sb[:, 0, :], win(0, 0, 0), True, False))
                    # Interleave y1 (cols 0:64) and y2 (cols 64:128) edge taps:
                    # the PE overlaps adjacent matmuls on disjoint column
                    # groups, doubling M=64 throughput.
                    for mi, m in enumerate((-2, -1, 1, 2)):
                        for k in range(2):
                            mms.append((P_cur[0:64, :], w64_sb[:, 2 * mi + k, :], win(k, 0, 2 * m), False, False))
                            mms.append((P_cur[64:128, :], w64_sb[:, 8 + 2 * mi + k, :], win(k, 2 * m, 0), False, False))
                    mms.append((P_cur[:, :], w128_sb[:, 1, :], win(1, 0, 0), False, True))
                else:
                    mms.append((P_cur[:, :], w128_sb[:, 6, :], win(0, 0, 0), True, False))
                    for m in (-2, -1, 1, 2):
                        t_w = 2 + 2 * (m + 2)
                        for k in range(2):
                            mms.append((P_cur[:, :], w128_sb[:, t_w + k, :], win(k, 2 * m, -2 * m), False, False))
                    mms.append((P_cur[:, :], w128_sb[:, 7, :], win(1, 0, 0), False, True))
                return mms

            def group_waits(g, t, br, cur_stripe):
                # wait list gating group g; stripe covers max row-shift reach
                ws = []
                s_needed = row2stripe[min(4 * t + TR - 1 + RP, H - 1)]
                while cur_stripe < s_needed:
                    cur_stripe += 1
                    ws.append((x_sems[cur_stripe], 32))
                if g >= NPSUM and br == 0:
                    # one wait per tile-pair covers both groups (7 banks slack)
                    ws.append((act_sem, g - NPSUM + 2))
                return ws, cur_stripe

            cur_stripe = -1
            pending = None  # last matmul of the previous group (carries inc)
            for g, (t, br) in enumerate(groups):
                P_cur = psum[g % NPSUM]
                waits, cur_stripe = group_waits(g, t, br, cur_stripe)
                if pending is None:
                    for sem, v in waits:
                        tensor.wait_ge(sem, v)
                else:
                    # evaluate this group's waits while the previous group's
                    # final matmul still streams, then retire that matmul
                    o, l, r, st, sp = pending
                    for sem, v in waits:
                        tensor.wait_ge(sem, v)
                    tensor.matmul(o, lhsT=l, rhs=r, start=st, stop=sp).then_inc(mm_sem, 1)
                mms = group_mms(t, br)
                for o, l, r, st, sp in mms[:-1]:
                    tensor.matmul(o, lhsT=l, rhs=r, start=st, stop=sp)
                pending = mms[-1]
            o, l, r, st, sp = pending
            tensor.matmul(o, lhsT=l, rhs=r, start=st, stop=sp).then_inc(mm_sem, 1)

        @block.scalar
        def _(scalar):
            scalar.dma_start(w128_sb[:, :, :], w128_ext[:, :, :]).then_inc(w_sem, 16)
            scalar.dma_start(w64_sb[:, :, :], w64_ext[:, :, :]).then_inc(w_sem, 16)
            scalar.dma_start(bias_sb[:, :], bias_ext[:]).then_inc(w_sem, 16)
            scalar.wait_ge(w_sem, 48)  # bias loaded
            for g, (t, br) in enumerate(groups):
                i0 = 4 * t
                slot = g % NSTAGE
                scalar.wait_ge(mm_sem, g + 1)
                if g >= NSTAGE:
                    # all previous users of this stage slot fully drained
                    scalar.wait_ge(o_sems[slot], 16 * (g // NSTAGE))
                st = stage[slot]
                scalar.activation(
                    st[:, :],
                    psum[g % NPSUM][:, :],
                    mybir.ActivationFunctionType.Identity,
                    bias=bias_sb[:, br : br + 1],
                ).then_inc(act_sem, 1)
                # HWDGE transfer reads SBUF async; wait for the activation's
                # writeback before issuing the drain DMA.
                scalar.wait_ge(act_sem, g + 1)
                scalar.dma_start(
                    out_ext[128 * br : 128 * (br + 1), i0 : i0 + TR, :],
                    st[:].rearrange("p (r w) -> p r w", r=TR),
                ).then_inc(o_sems[slot], 16)
            for slot in range(NSTAGE):
                scalar.wait_ge(o_sems[slot], 16 * (len(groups) // NSTAGE))

    return nc


def kernel(x, w_h, b_h, w_v, b_v, w_d1, b_d1, w_d2, b_d2):
    global _cached_nc, LAST_EXEC_TIME_NS, LAST_RESULTS
    x = np.asarray(x, dtype=np.float32)
    w128, w64, bias = _build_weights(
        np.asarray(w_h, np.float32), np.asarray(b_h, np.float32),
        np.asarray(w_v, np.float32), np.asarray(b_v, np.float32),
        np.asarray(w_d1, np.float32), np.asarray(b_d1, np.float32),
        np.asarray(w_d2, np.float32), np.asarray(b_d2, np.float32),
    )
    w128b = w128.astype(NPDT)
    w64b = w64.astype(NPDT)
    xp = np.zeros((B, C, H, WP), np.float32)
    xp[:, :, :, CP : CP + W] = x
    xb = xp.astype(NPDT)

    if _cached_nc is None:
        _cached_nc = _build_nc()
    nc = _cached_nc

    in_maps = [
        {"x": xb[b], "w128": w128b, "w64": w64b, "bias": bias}
        for b in range(NCORES)
    ]
    trace = bool(os.environ.get("KERNEL_TRACE"))
    res = run_bass_kernel_spmd(nc, in_maps, core_ids=list(range(NCORES)), trace=trace)
    LAST_RESULTS = res
    LAST_EXEC_TIME_NS = res.exec_time_ns
    out = np.stack([res.results[i]["out"] for i in range(NCORES)], axis=0)
    return out.astype(np.float32)


# revision 30
# speedup vs baseline: 1.1899x; 1.1899x over previous
"""Trainium2 Bass kernel for nn_Asterisk: 4-branch directional dilated conv.

Math: every branch is a sum of 5 shifted 1x1 convolutions over C=256:
  y1[i,j] = sum_m Wh[m]  . x[i,      j+2m]   (horizontal)
  y2[i,j] = sum_m Wv[m]  . x[i+2m,   j]      (vertical)
  y3[i,j] = sum_m Wd1[m] . x[i+2m, j-2m]     (shear-diag)
  y4[i,j] = sum_m Wd2[2-m]. x[i+2m, j-2m]    (shear-antidiag; same taps as y3)
All shifts are zero-padded at image boundaries.

Strategy: data-parallel over batch (B=8 -> 8 NeuronCores, zero collectives).
Per core: x[b] lives in SBUF as two 128-channel chunks, rows zero-padded by
4 top/bottom (memset) and columns zero-padded by 4 (host-side, so every DMA
stays contiguous) -- shifted taps then need no boundary handling at all.
Each output tile = 4 rows x 128 cols (N=512, one PSUM bank),
accumulated via matmuls over (tap, chunk). y3/y4 share rhs tiles so their
weights concat to full M=128; y1/y2 share only the center tap.
Compute dtype bf16 (host-cast), accumulation fp32.
"""
import os
import sys

for _p in ("/opt/trn_rl_repo",):
    if _p not in sys.path:
        sys.path.insert(0, _p)

import numpy as np
import ml_dtypes

import concourse.bass as bass
import concourse.mybir as mybir
from concourse.bass_utils import run_bass_kernel_spmd

B, C, H, W = 8, 256, 128, 128
OC = C // 4          # 64 output channels per branch
NCORES = 8
RP = 4               # row padding (max |shift|)
CP = 4               # column padding (host-side, in DRAM)
HP = H + 2 * RP      # padded rows in SBUF
WP = W + 2 * CP      # padded cols (host-padded)
TR = 4               # output rows per tile
NT = H // TR         # 32 row-tiles
NPSUM = 8            # psum banks in flight
NSTAGE = 4           # sbuf staging buffers for output

DT = mybir.dt.bfloat16
NPDT = ml_dtypes.bfloat16

LAST_EXEC_TIME_NS = None
LAST_RESULTS = None

_cached_nc = None


def _build_weights(w_h, b_h, w_v, b_v, w_d1, b_d1, w_d2, b_d2):
    """Build the stationary matmul operands.

    Returns:
      w128: [128, 12, 128]  (c-within-chunk, tile, M) tiles with M=128:
            t=0,1: center tap [y1|y2] for chunk k=t
            t=2+2*(m+2)+k: diag tap m for chunk k, [y3|y4]
      w64:  [128, 16, 64] tiles with M=64:
            t=2*mi+k: y1 edge tap (m in -2,-1,1,2), chunk k
            t=8+2*mi+k: y2 edge tap, chunk k
      bias: [128, 2] f32, col0=[b_h;b_v], col1=[b_d1;b_d2]
    """
    ms_edge = (-2, -1, 1, 2)
    w128 = np.zeros((128, 12, 128), np.float32)
    w64 = np.zeros((128, 16, 64), np.float32)
    for k in range(2):
        cs = slice(128 * k, 128 * (k + 1))
        # center: y1 tap m=0 is w_h[..., 2]; y2 tap m=0 is w_v[..., 2, 0]
        w128[:, k, 0:64] = w_h[:, cs, 0, 2].T
        w128[:, k, 64:128] = w_v[:, cs, 2, 0].T
        for mi, m in enumerate((-2, -1, 0, 1, 2)):
            t = 2 + 2 * (m + 2) + k
            w128[:, t, 0:64] = w_d1[:, cs, m + 2, 0].T
            w128[:, t, 64:128] = w_d2[:, cs, 0, 2 - m].T
        for mi, m in enumerate(ms_edge):
            w64[:, 2 * mi + k, :] = w_h[:, cs, 0, m + 2].T
            w64[:, 8 + 2 * mi + k, :] = w_v[:, cs, m + 2, 0].T
    bias = np.stack(
        [np.concatenate([b_h, b_v]), np.concatenate([b_d1, b_d2])], axis=1
    ).astype(np.float32)
    return w128, w64, bias


def _build_nc():
    nc = bass.Bass()
    x_ext = nc.declare_dram_parameter("x", [C, H, WP], DT, isOutput=False)
    w128_ext = nc.declare_dram_parameter("w128", [128, 12, 128], DT, isOutput=False)
    w64_ext = nc.declare_dram_parameter("w64", [128, 16, 64], DT, isOutput=False)
    bias_ext = nc.declare_dram_parameter("bias", [128, 2], mybir.dt.float32, isOutput=False)
    out_ext = nc.declare_dram_parameter("out", [C, H, W], mybir.dt.float32, isOutput=True)

    xsb = [nc.alloc_sbuf_tensor(f"x{k}", [128, HP, WP], DT).ap() for k in range(2)]
    w128_sb = nc.alloc_sbuf_tensor("w128s", [128, 12, 128], DT).ap()
    w64_sb = nc.alloc_sbuf_tensor("w64s", [128, 16, 64], DT).ap()
    bias_sb = nc.alloc_sbuf_tensor("biass", [128, 2], mybir.dt.float32).ap()
    stage = [
        nc.alloc_sbuf_tensor(f"st{i}", [128, TR * W], mybir.dt.float32).ap()
        for i in range(NSTAGE)
    ]
    psum = [
        nc.alloc_psum_tensor(f"p{i}", [128, TR * W], mybir.dt.float32).ap()
        for i in range(NPSUM)
    ]
    warm = nc.alloc_sbuf_tensor("warm", [128, 512], DT).ap()

    # input stripes (row0, nrows): fine-grained early so compute starts ASAP
    STRIPES = [(0, 8), (8, 8), (16, 16), (32, 16), (48, 16), (64, 32), (96, 32)]
    NSTRIPE = len(STRIPES)
    row2stripe = {}
    for si, (sr0, sn) in enumerate(STRIPES):
        for r in range(sr0, sr0 + sn):
            row2stripe[r] = si
    NWARM = 16  # PE warm-up matmuls (HAM ramp + queue priming) during DMA wait

    # group list: g = 2*t + br, br 0 => [y1|y2] channels 0:128, 1 => [y3|y4]
    groups = [(t, br) for t in range(NT) for br in (0, 1)]

    import contextlib
    with contextlib.ExitStack() as _stack:
        block = _stack.enter_context(nc.Block())
        w_sem = _stack.enter_context(nc.semaphore("w_sem"))
        ms_sem = _stack.enter_context(nc.semaphore("ms_sem"))
        mm_sem = _stack.enter_context(nc.semaphore("mm_sem"))
        act_sem = _stack.enter_context(nc.semaphore("act_sem"))
        x_sems = [_stack.enter_context(nc.semaphore(f"x_sem{s}")) for s in range(NSTRIPE)]
        o_sems = [_stack.enter_context(nc.semaphore(f"o_sem{i}")) for i in range(NSTAGE)]

        @block.sync
        def _(sync):
            for s, (r0, nr) in enumerate(STRIPES):
                for k in range(2):
                    sync.dma_start(
                        xsb[k][:, RP + r0 : RP + r0 + nr, :],
                        x_ext[128 * k : 128 * (k + 1), r0 : r0 + nr, :],
                    ).then_inc(x_sems[s], 16)

        @block.vector
        def _(vector):
            vector.memset(warm[:, :], 0.0).then_inc(ms_sem, 1)
            for k in range(2):
                vector.memset(xsb[k][:, 0:RP, :], 0.0).then_inc(ms_sem, 1)
                vector.memset(xsb[k][:, RP + H : HP, :], 0.0).then_inc(ms_sem, 1)

        @block.tensor
        def _(tensor):
            tensor.wait_ge(ms_sem, 1)
            # Warm-up: keeps the PE HAM busy during the input-DMA wait so the
            # real matmuls all run at 2.4 GHz; results are never read.
            for wi in range(NWARM):
                tensor.matmul(psum[wi % NPSUM][:, :], lhsT=warm[:, 0:128], rhs=warm[:, :],
                              start=True, stop=True)
            tensor.wait_ge(ms_sem, 5)
            tensor.wait_ge(w_sem, 48)

            def win_f(r0):
                def win(k, di, dj):
                    return xsb[k][:, r0 + di : r0 + di + TR, CP + dj : CP + dj + W]
                return win

            def group_mms(t, br):
                # (out_ap, lhsT_ap, rhs_ap, start, stop). First and last
                # matmul of each group are full-coverage (PSUM accumulation
                # group start/stop applies to whole byte ranges): center-tap
                # chunk 0 opens with start=True, chunk 1 closes with stop=True.
                win = win_f(RP + 4 * t)
                mms = []
                if br == 0:
                    mms.append((P_cur[:, :], w128_sb[:, 0, :], win(0, 0, 0), True, False))
                    # Interleave y1 (cols 0:64) and y2 (cols 64:128) edge taps:
                    # the PE overlaps adjacent matmuls on disjoint column
                    # groups, doubling M=64 throughput.
                    for mi, m in enumerate((-2, -1, 1, 2)):
                        for k in range(2):
                            mms.append((P_cur[0:64, :], w64_sb[:, 2 * mi + k, :], win(k, 0, 2 * m), False, False))
                            mms.append((P_cur[64:128, :], w64_sb[:, 8 + 2 * mi + k, :], win(k, 2 * m, 0), False, False))
                    mms.append((P_cur[:, :], w128_sb[:, 1, :], win(1, 0, 0), False, True))
                else:
                    mms.append((P_cur[:, :], w128_sb[:, 6, :], win(0, 0, 0), True, False))
                    for m in (-2, -1, 1, 2):
                        t_w = 2 + 2 * (m + 2)
                        for k in range(2):
                            mms.append((P_cur[:, :], w128_sb[:, t_w + k, :], win(k, 2 * m, -2 * m), False, False))
                    mms.append((P_cur[:, :], w128_sb[:, 7, :], win(1, 0, 0), False, True))
                return mms

            def group_waits(g, t, br, cur_stripe):
                # wait list gating group g; stripe covers max row-shift reach
                ws = []
                s_needed = row2stripe[min(4 * t + TR - 1 + RP, H - 1)]
                while cur_stripe < s_needed:
                    cur_stripe += 1
                    ws.append((x_sems[cur_stripe], 32))
                if g >= NPSUM and br == 0:
                    # one wait per tile-pair covers both groups (7 banks slack)
                    ws.append((act_sem, g - NPSUM + 2))
                return ws, cur_stripe

            cur_stripe = -1
            pending = None  # last matmul of the previous group (carries inc)
            for g, (t, br) in enumerate(groups):
                P_cur = psum[g % NPSUM]
                waits, cur_stripe = group_waits(g, t, br, cur_stripe)
                if pending is None:
                    for sem, v in waits:
                        tensor.wait_ge(sem, v)
                else:
                    # evaluate this group's waits while the previous group's
                    # final matmul still streams, then retire that matmul
                    o, l, r, st, sp = pending
                    for sem, v in waits:
                        tensor.wait_ge(sem, v)
                    tensor.matmul(o, lhsT=l, rhs=r, start=st, stop=sp).then_inc(mm_sem, 1)
                mms = group_mms(t, br)
                for o, l, r, st, sp in mms[:-1]:
                    tensor.matmul(o, lhsT=l, rhs=r, start=st, stop=sp)
                pending = mms[-1]
            o, l, r, st, sp = pending
            tensor.matmul(o, lhsT=l, rhs=r, start=st, stop=sp).then_inc(mm_sem, 1)

        @block.scalar
        def _(scalar):
            scalar.dma_start(w128_sb[:, :, :], w128_ext[:, :, :]).then_inc(w_sem, 16)
            scalar.dma_start(w64_sb[:, :, :], w64_ext[:, :, :]).then_inc(w_sem, 16)
            scalar.dma_start(bias_sb[:, :], bias_ext[:]).then_inc(w_sem, 16)
            scalar.wait_ge(w_sem, 48)  # bias loaded
            for g, (t, br) in enumerate(groups):
                i0 = 4 * t
                slot = g % NSTAGE
                scalar.wait_ge(mm_sem, g + 1)
                if g >= NSTAGE:
                    # all previous users of this stage slot fully drained
                    scalar.wait_ge(o_sems[slot], 16 * (g // NSTAGE))
                st = stage[slot]
                scalar.activation(
                    st[:, :],
                    psum[g % NPSUM][:, :],
                    mybir.ActivationFunctionType.Identity,
                    bias=bias_sb[:, br : br + 1],
                ).then_inc(act_sem, 1)
                # HWDGE transfer reads SBUF async; wait for the activation's
                # writeback before issuing the drain DMA.
                scalar.wait_ge(act_sem, g + 1)
                scalar.dma_start(
                    out_ext[128 * br : 128 * (br + 1), i0 : i0 + TR, :],
                    st[:].rearrange("p (r w) -> p r w", r=TR),
                ).then_inc(o_sems[slot], 16)
            for slot in range(NSTAGE):
                scalar.wait_ge(o_sems[slot], 16 * (len(groups) // NSTAGE))

    return nc


def kernel(x, w_h, b_h, w_v, b_v, w_d1, b_d1, w_d2, b_d2):
    global _cached_nc, LAST_EXEC_TIME_NS, LAST_RESULTS
    x = np.asarray(x, dtype=np.float32)
    w128, w64, bias = _build_weights(
        np.asarray(w_h, np.float32), np.asarray(b_h, np.float32),
        np.asarray(w_v, np.float32), np.asarray(b_v, np.float32),
        np.asarray(w_d1, np.float32), np.asarray(b_d1, np.float32),
        np.asarray(w_d2, np.float32), np.asarray(b_d2, np.float32),
    )
    w128b = w128.astype(NPDT)
    w64b = w64.astype(NPDT)
    xp = np.zeros((B, C, H, WP), np.float32)
    xp[:, :, :, CP : CP + W] = x
    xb = xp.astype(NPDT)

    if _cached_nc is None:
        _cached_nc = _build_nc()
    nc = _cached_nc

    in_maps = [
        {"x": xb[b], "w128": w128b, "w64": w64b, "bias": bias}
        for b in range(NCORES)
    ]
    trace = bool(os.environ.get("KERNEL_TRACE"))
    res = run_bass_kernel_spmd(nc, in_maps, core_ids=list(range(NCORES)), trace=trace)
    LAST_RESULTS = res
    LAST_EXEC_TIME_NS = res.exec_time_ns
    out = np.stack([res.results[i]["out"] for i in range(NCORES)], axis=0)
    return out.astype(np.float32)
